# revision 28
# baseline (speedup 1.0000x reference)
"""Trainium2 Bass kernel for nn_BartPooler_53815940219079 (segment_reduce).

Computes, for each of B*T segments of a [B, S, H] hidden-state tensor:
  feat = concat([segment_max, segment_mean])  -> tanh(feat @ W.T + b)

Strategy (8 NeuronCores, SPMD — one program, per-core data):
  * Host compacts each segment's used tokens into a per-core fp16 token
    stream, padding every segment with duplicates of its first token so that
    each segment occupies a whole number of G=4-token "groups" (plus
    pure-duplicate groups whose negative membership weight cancels the
    padding in the sum — exact even in fp16 since t0+t0 doubles losslessly).
    Segments are dealt snake-wise across cores by size, largest first, and
    per-slot group counts are quantized into runs of equal length so the
    per-segment max becomes one batched reduce per run.
  * Device, per 128-group tile: the grouped max tree and the first sum level
    run on VectorE (fp16); tokens 2/3 feed TensorE raw, so segment means
    accumulate on the PE directly in transposed [h, slot] layout (membership
    weights carry MSCALE/cnt; the mean half of W is pre-divided by MSCALE to
    keep fp16 weights in normal range).  PE transposes the max partials,
    VectorE does one batched run-reduce per covered run, and the epilogue is
    a fused [2H] x [2H, D] fp16 GEMM (4-up PE column packing + fp16 fold)
    with bias + tanh in fp32.  hid tiles alternate between the sync and
    scalar HWDGE queues (~180 GB/s each) with W chunks paced behind them.
"""

import numpy as np

import concourse.bacc as bacc
import concourse.mybir as mybir
import concourse.tile as tile
from concourse.bass_utils import run_bass_kernel_spmd
from concourse.masks import make_identity
from concourse.tile import add_dep_helper

NCORES = 8
G = 4          # tokens per group
PTILE = 128 * G  # tokens per main tile

B, S, H, T = 16, 4096, 1024, 16
D_OUT = 1024
HB = H // 128  # h-blocks per hidden vector

F32 = mybir.dt.float32
F16 = mybir.dt.float16
MSCALE = 16.0  # mean-path scale: member *= MSCALE, W mean-rows /= MSCALE


def _build_schedule(parts, turns):
    """Host-side: segment list -> per-core compacted layout (uniform shapes)."""
    Bn, Tn = parts.shape
    segs = []  # (global_row, example, start_token, count)
    for b in range(Bn):
        cum = 0
        for j in range(Tn):
            c = int(parts[b, j])
            if j < int(turns[b]):
                segs.append((b * Tn + j, b, 1 + cum, c))
            cum += c

    # Deal segments to cores by size rank: slot j holds the 8 segments of
    # ranks [8j, 8j+8), one per core, so the uniform per-slot group count
    # L[j] (max over cores) is as tight as possible.
    order = sorted(range(len(segs)), key=lambda i: -segs[i][3])
    core_slots = [[] for _ in range(NCORES)]
    for rank, i in enumerate(order):
        core_slots[rank % NCORES].append(segs[i])
    # Largest segments first: their long run-reduces hide inside the DMA
    # stream, and the final tile covers only one small batched run.
    seg_cap = max(len(s) for s in core_slots)

    def groups_needed(cnt):
        g = (cnt + G - 1) // G
        if cnt % G:
            g += 1  # at least one pure-duplicate group for the compensation
        return g

    # Uniform per-slot group counts across cores.
    L = []
    for j in range(seg_cap):
        m = 1
        for c in range(NCORES):
            if j < len(core_slots[c]):
                m = max(m, groups_needed(core_slots[c][j][3]))
        L.append(m)
    # Quantize L into runs of consecutive equal values (slots are sorted by
    # size, so padding is small) so the per-segment max becomes one
    # tensor_reduce per run instead of one per slot.  DP balances padding
    # (~70ns/group: DMA + tree) against per-reduce overhead (~230ns).
    W_PAD, W_RUN = 70.0, 500.0
    n = len(L)
    best = [0.0] * (n + 1)
    cut = [0] * (n + 1)
    for j in range(1, n + 1):
        b = None
        for i in range(j):
            lmax = max(L[i:j])
            c = best[i] + W_RUN + W_PAD * sum(lmax - x for x in L[i:j])
            if b is None or c < b:
                b, cut[j] = c, i
        best[j] = b
    runs = []  # (j0, j1_exclusive, Lrun)
    j = n
    while j > 0:
        i = cut[j]
        runs.append((i, j, max(L[i:j])))
        j = i
    runs.reverse()
    # force a run boundary at the GEMM slot split so the A half's reduces
    # never wait on a straddling run
    SPLIT = n // 2
    split_runs = []
    for (i, j, lr) in runs:
        if i < SPLIT < j:
            split_runs.append((i, SPLIT, lr))
            split_runs.append((SPLIT, j, lr))
        else:
            split_runs.append((i, j, lr))
    runs = split_runs
    for (i, j, lr) in runs:
        for k in range(i, j):
            L[k] = lr
    A = np.concatenate([[0], np.cumsum(L)]).astype(np.int64)  # slot -> group start
    ngroups = int(A[-1])
    ntiles = (ngroups + 127) // 128
    ntok = ngroups * G

    # Per-core token-gather indices (into flat [B*S]) and membership weights.
    tok_idx = np.full((NCORES, ntok), -1, dtype=np.int64)
    member = np.zeros((NCORES, 128, ntiles, seg_cap), dtype=np.float32)
    out_map = np.full((NCORES, seg_cap), -1, dtype=np.int64)
    for c in range(NCORES):
        for j, (grow, b, s0, cnt) in enumerate(core_slots[c]):
            out_map[c, j] = grow
            g0 = int(A[j])
            nfull, rem = divmod(cnt, G)
            base = b * S + s0
            t0 = base  # first token, used as the harmless duplicate
            pos = g0 * G
            tok_idx[c, pos:pos + cnt] = np.arange(base, base + cnt)
            pos += cnt
            npure = L[j] - nfull - (1 if rem else 0)
            r = (G - rem) % G
            if r:
                tok_idx[c, pos:pos + r] = t0
                pos += r
            if npure:
                tok_idx[c, pos:pos + npure * G] = t0
            # weights: real groups 1/cnt, pure groups -r/(npure*G*cnt)
            inv = 1.0 / cnt
            nreal = nfull + (1 if rem else 0)
            for k in range(nreal):
                g = g0 + k
                member[c, g % 128, g // 128, j] = inv
            beta = -r / (npure * G) * inv if (npure and r) else 0.0
            for k in range(npure):
                g = g0 + nreal + k
                member[c, g % 128, g // 128, j] = beta
    return {
        "core_slots": core_slots,
        "seg_cap": seg_cap,
        "L": L,
        "A": A,
        "runs": runs,
        "ntiles": ntiles,
        "ntok": ntok,
        "tok_idx": tok_idx,
        "member": member,
        "out_map": out_map,
        "nrows": Bn * Tn,
    }


def _build_program(ntiles, seg_cap, A, L, runs, split=False, stage_only=True):
    """Emit the SPMD Bass program (identical for all cores)."""
    ngroups = int(A[-1])
    ntok = ngroups * G

    nc = bacc.Bacc("TRN2", target_bir_lowering=False, debug=False,
                   num_devices=NCORES)
    hid = nc.dram_tensor("hid", [ntok, H], F16, kind="ExternalInput")
    mem = nc.dram_tensor("mem", [128, ntiles, seg_cap], F16, kind="ExternalInput")
    wt = nc.dram_tensor("wt", [2 * H, D_OUT], F16, kind="ExternalInput")
    brep = nc.dram_tensor("brep", [seg_cap, D_OUT], F32, kind="ExternalInput")
    fold = nc.dram_tensor("fold", [128, seg_cap], F16, kind="ExternalInput")
    out = nc.dram_tensor("out", [seg_cap, D_OUT], F32, kind="ExternalOutput")

    # each run's per-segment max reduce is emitted right after the last
    # tile covering its group range
    cover = [[] for _ in range(ntiles)]
    for (j0, j1, lr) in runs:
        cover[(int(A[j1]) - 1) // 128].append((j0, j1, lr))
    # per-tile trmax column ranges needed by straddling runs (merged)
    stage = [[] for _ in range(ntiles)]
    for (j0, j1, lr) in runs:
        a, b = int(A[j0]), int(A[j0]) + (j1 - j0) * lr
        if a // 128 != (b - 1) // 128:  # straddles a tile boundary
            for t in range(a // 128, (b - 1) // 128 + 1):
                c0, c1 = max(a, t * 128), min(b, (t + 1) * 128)
                stage[t].append((c0, c1))
    for t in range(ntiles):
        merged = []
        for (c0, c1) in sorted(stage[t]):
            if merged and c0 <= merged[-1][1]:
                merged[-1] = (merged[-1][0], max(merged[-1][1], c1))
            else:
                merged.append((c0, c1))
        stage[t] = merged
    if not stage_only:  # full-tile staging (previous behavior)
        stage = [[(t * 128, min((t + 1) * 128, ngroups))]
                 for t in range(ntiles)]

    with tile.TileContext(nc) as tc:
        with (
            tc.tile_pool(name="const", bufs=1) as constp,
            tc.tile_pool(name="hidp", bufs=5) as hidp,
            tc.tile_pool(name="partial", bufs=3) as partp,
            tc.tile_pool(name="psum_tr", bufs=2, space="PSUM") as trpp,
            tc.tile_pool(name="psum_acc", bufs=1, space="PSUM") as accp,
            tc.tile_pool(name="psum_gem", bufs=2, space="PSUM") as gemp,
            tc.tile_pool(name="small", bufs=1) as smallp,
        ):
            ident = constp.tile([128, 128], F16)
            make_identity(nc, ident[:])

            # W on the scalar-engine HWDGE queue, in 0.5MB chunks paced
            # behind the per-tile hid streams (sync queue) so the loop's
            # tile supply isn't starved of DMA bandwidth.
            wt_sb = constp.tile([128, 2 * HB, D_OUT], F16)
            wt_view = wt[:].rearrange("(kb p) n -> p kb n", p=128)
            wt_dmas = []
            for wch in range(HB):
                weng = nc.scalar if wch % 2 == 0 else nc.sync
                wt_dmas.append(weng.dma_start(
                    out=wt_sb[:, 2 * wch:2 * wch + 2, :],
                    in_=wt_view[:, 2 * wch:2 * wch + 2, :],
                ))
            halfc = seg_cap // 2 if split else seg_cap
            brepA_sb = constp.tile([halfc, D_OUT], F32)
            nc.scalar.dma_start(out=brepA_sb[:], in_=brep[:halfc, :])
            if split:
                brepB_sb = constp.tile([halfc, D_OUT], F32)
                nc.scalar.dma_start(out=brepB_sb[:], in_=brep[halfc:, :])
            fold_sb = constp.tile([128, seg_cap], F16)
            nc.scalar.dma_start(out=fold_sb[:], in_=fold[:])
            mem_sb = constp.tile([128, ntiles, seg_cap], F16)
            nc.sync.dma_start(out=mem_sb[:], in_=mem[:])

            trmax = constp.tile([128, HB, ngroups], F16)
            maxT = smallp.tile([128, HB, seg_cap], F16)
            # Two mean accumulators (slot halves), each padded to a full
            # 2KB PSUM bank so their start=True zeroings stay independent.
            # The A half (big slots, done mid-stream) feeds an early GEMM.
            mean_psA = accp.tile([128, HB, 64], F32, tag="accA")
            mean_psB = (accp.tile([128, HB, 64], F32, tag="accB",
                                  name="mean_psB")
                        if split else None)
            tA_mean = (int(A[halfc]) - 1) // 128 if split else ntiles - 1
            tB0 = int(A[halfc]) // 128 if split else ntiles

            def emit_gemm_half(tag, s0, mean_psX, brep_sb):
                m = halfc
                meansT = smallp.tile([128, HB, m], F16, name=f"meansT{tag}")
                nc.scalar.copy(out=meansT[:], in_=mean_psX[:, :, :m])
                osb = smallp.tile([m, D_OUT], F32, name=f"osb{tag}")
                for nh in range(2):
                    nsl = slice(nh * 512, (nh + 1) * 512)
                    gem_ps = gemp.tile([128, 512], F32, tag="gem")
                    for i in range(4):
                        for cg in range(4):
                            kb = 2 * cg + i if i < 2 else HB + 2 * cg + i - 2
                            lhsT = (maxT[:, kb, s0:s0 + m] if kb < HB
                                    else meansT[:, kb - HB, :])
                            nc.tensor.matmul(
                                gem_ps[32 * cg:32 * cg + m, :],
                                lhsT=lhsT,
                                rhs=wt_sb[:, kb, nsl],
                                start=(i == 0),
                                stop=(i == 3),
                                tile_position=(0, 32 * cg),
                            )
                    gem_sb = smallp.tile([128, 512], F16, name=f"gsb{tag}{nh}")
                    nc.scalar.copy(out=gem_sb[:], in_=gem_ps[:])
                    fold_ps = trpp.tile([m, 512], F32, tag="fold")
                    nc.tensor.matmul(fold_ps[:], lhsT=fold_sb[:, :m],
                                     rhs=gem_sb[:], start=True, stop=True)
                    nc.vector.tensor_add(out=osb[:, nsl], in0=fold_ps[:],
                                         in1=brep_sb[:, nsl])
                    nc.scalar.activation(osb[:, nsl], osb[:, nsl],
                                         mybir.ActivationFunctionType.Tanh)
                nc.sync.dma_start(out=out[s0:s0 + m, :], in_=osb[:])

            for t in range(ntiles):
                pt = min(128, ngroups - t * 128)  # groups in this tile
                ht = hidp.tile([128, G * H], F16)
                heng = nc.sync if t % 2 == 0 else nc.scalar
                hdma = heng.dma_start(
                    out=ht[:pt, :],
                    in_=hid[t * PTILE:t * PTILE + pt * G, :]
                        .rearrange("(p g) h -> p (g h)", g=G),
                )
                if t < len(wt_dmas):
                    add_dep_helper(wt_dmas[t].ins, hdma.ins, True,
                                   "pace W chunks behind hid tiles")
                # Max tree first (transposes + reduces unblock earliest),
                # then the full sum tree for the mean matmul.
                gmax = partp.tile([128, H], F16, tag="gmax")
                mx2 = partp.tile([128, 2 * H], F16, tag="mx2")
                tsm = partp.tile([128, H], F16, tag="tsm")
                half = G // 2 * H
                nc.vector.tensor_tensor(out=mx2[:pt], in0=ht[:pt, :half],
                                        in1=ht[:pt, half:], op=mybir.AluOpType.max)
                nc.vector.tensor_tensor(out=gmax[:pt], in0=mx2[:pt, :H],
                                        in1=mx2[:pt, H:], op=mybir.AluOpType.max)
                nc.vector.tensor_tensor(out=tsm[:pt], in0=ht[:pt, :H],
                                        in1=ht[:pt, H:2 * H], op=mybir.AluOpType.add)
                # Segment means accumulate on PE directly in [h, slot]
                # layout: meansT[h, j] += sum_g lhs[g, h] * member[g, j]
                # (weights already carry 1/cnt).  Vector pre-folds only
                # tokens 0+1; tokens 2 and 3 feed the PE raw, trading two
                # extra N=32 matmuls per chunk for a 1024-col vector op.
                # start=True zeroes the whole 2KB PSUM bank (all 8 chunk
                # regions), so only the very first matmul may carry it.
                # transpose the max partials first ([group, h] -> [h,
                # group]): the max chain (copy + run reduces) hangs off them
                trp = trpp.tile([128, H], F16, tag="trp")
                for hb in range(HB):
                    nc.tensor.transpose(
                        trp[:, hb * 128:hb * 128 + pt],
                        gmax[:pt, hb * 128:(hb + 1) * 128],
                        ident[:pt, :pt],
                    )
                for c in range(HB):
                    lhss = (tsm[:pt, c * 128:(c + 1) * 128],
                            ht[:pt, 2 * H + c * 128:2 * H + (c + 1) * 128],
                            ht[:pt, 3 * H + c * 128:3 * H + (c + 1) * 128])
                    for v, lhsT in enumerate(lhss):
                        if t <= tA_mean:
                            nc.tensor.matmul(
                                mean_psA[:, c, :halfc],
                                lhsT=lhsT,
                                rhs=mem_sb[:pt, t, :halfc],
                                start=(t == 0 and c == 0 and v == 0),
                                stop=(t == tA_mean and c == HB - 1 and v == 2),
                            )
                        if t >= tB0:
                            nc.tensor.matmul(
                                mean_psB[:, c, :halfc],
                                lhsT=lhsT,
                                rhs=mem_sb[:pt, t, halfc:],
                                start=(t == tB0 and c == 0 and v == 0),
                                stop=(t == ntiles - 1 and c == HB - 1 and v == 2),
                            )
                # stage into trmax only the columns a straddling run will
                # read; contained runs reduce straight from the PSUM
                for (c0, c1) in stage[t]:
                    o0, o1 = c0 - t * 128, c1 - t * 128
                    nc.scalar.copy(
                        out=trmax[:, :, c0:c1],
                        in_=trp[:].rearrange("p (b g) -> p b g", g=128)
                            [:, :, o0:o1],
                    )
                # per-segment max for slot runs fully covered by now;
                # runs contained in this tile read the transpose PSUM
                # directly, skipping the trmax staging copy's latency
                for (j0, j1, lr) in cover[t]:
                    a = int(A[j0])
                    span = (j1 - j0) * lr
                    if a >= t * 128 and a + span <= t * 128 + pt:
                        off = a - t * 128
                        src_ap = (trp[:]
                                  .rearrange("p (b g) -> p b g", g=128)
                                  [:, :, off:off + span])
                    else:
                        src_ap = trmax[:, :, a:a + span]
                    nc.vector.reduce_max(
                        out=maxT[:, :, j0:j1],
                        in_=src_ap.rearrange("p b (n l) -> p b n l", l=lr),
                        axis=mybir.AxisListType.X,
                    )

            emit_gemm_half("A", 0, mean_psA, brepA_sb)
            if split:
                emit_gemm_half("B", halfc, mean_psB, brepB_sb)

    nc.compile()
    return nc


def _build_in_maps(sched, hidden_states, W, b):
    seg_cap, ntiles = sched["seg_cap"], sched["ntiles"]
    flat = np.ascontiguousarray(
        np.asarray(hidden_states, dtype=np.float32)).reshape(B * S, H)
    wt_f = np.asarray(W, dtype=np.float32).T.copy()  # [2H, D]
    wt_f[H:] /= MSCALE  # mean-half rows absorb the member scale
    wt_np = np.ascontiguousarray(wt_f.astype(np.float16))
    brep_np = np.ascontiguousarray(
        np.broadcast_to(np.asarray(b, dtype=np.float32), (seg_cap, D_OUT)))
    fold_np = np.zeros((128, seg_cap), dtype=np.float16)
    for cg in range(4):
        for j in range(seg_cap):
            fold_np[32 * cg + j, j] = 1.0

    in_maps = []
    for c in range(NCORES):
        idx = sched["tok_idx"][c]
        stream = np.zeros((sched["ntok"], H), dtype=np.float16)
        valid = idx >= 0
        stream[valid] = flat[idx[valid]].astype(np.float16)
        memc = np.ascontiguousarray(
            (sched["member"][c].reshape(128, ntiles, seg_cap) * MSCALE)
            .astype(np.float16))
        in_maps.append({
            "hid": stream,
            "mem": memc,
            "wt": wt_np,
            "brep": brep_np,
            "fold": fold_np,
        })
    return in_maps


def kernel(hidden_states, W, b, turns, parts):
    parts = np.asarray(parts)
    turns = np.asarray(turns)

    sched = _build_schedule(parts, turns)
    nc = _build_program(sched["ntiles"], sched["seg_cap"],
                        sched["A"], sched["L"], sched["runs"])
    in_maps = _build_in_maps(sched, hidden_states, W, b)

    res = run_bass_kernel_spmd(nc, in_maps, list(range(NCORES)))

    full = np.zeros((sched["nrows"], D_OUT), dtype=np.float32)
    for c in range(NCORES):
        oc = res.results[c]["out"]
        for j in range(sched["seg_cap"]):
            g = sched["out_map"][c, j]
            if g >= 0:
                full[g] = oc[j]
    return full
